# revision 1
# baseline (speedup 1.0000x reference)
"""Trainium2 Bass kernel for nn_AdjointCircuitModule (13-qubit HEA circuit +
dense observable expectation), SPMD across 8 NeuronCores.

Strategy (v2)
-------------
loss = <psi|O|psi> = psi^T Osym psi, Osym = (O + O^T)/2.  Only the symmetric
part matters, so the host streams the upper triangle of S = O + O^T in
512x512 blocks: 136 blocks, 17 per core (every block identical cost =>
perfectly uniform SPMD program).  Blocks are fp16 (quantization error
~3e-4 on the scalar) => 8.9 MB/core vs 32 MB full-f32.  Same-row blocks
are paired so the matvec runs 1024-col moving operands.

* Circuit: every core simulates the full 13-qubit circuit redundantly, in
  fp16 (10-bit mantissa keeps |dpsi| ~1e-3; fp16 matmuls run at 2x the
  fp32r rate and get fast-weight-load).  State held as L0 matrix S[p, f]
  (qubits 0-6 on 128 partitions, 7-12 on 64 free cols).  Per layer:
    - weight tiles RAW = [RAre|RAim|-RAim|RAre] (128x512) and
      RBW = [RBre|RBdre|RBim|RBdim|-RBim|-RBdim|RBre|RBdre] (64x512)
      built from trig tables, pipelined 2 layers ahead of the state chain,
    - state chain: psA = 2 matmuls, PSUM->SBUF copy (2 engines), psB = 2
      matmuls (the _d columns compute the CX67 column-flip difference),
      CX67 = keep-part copy + 2 scalar_tensor_tensor.
* Matvec: per group (8 block-pairs + 1 single): PSUM [2,1024] accumulates
  4 matmuls (stationary = psi rows as fp16 pairs from W, moving = the fp16
  stream tile).  Drains go to SBUF vout; one DMA returns [2, 8704] and the
  host does the final block-dot against psi (psi_re/psi_im outputs).
"""

import math

import numpy as np

import concourse.bacc as bacc
import concourse.bass as bass
import concourse.mybir as mybir
import concourse.tile as tile
from concourse.bass_utils import run_bass_kernel_spmd

F32 = mybir.dt.float32
F32R = mybir.dt.float32r
F16 = mybir.dt.float16
AL = mybir.AluOpType

N_CORES = 8
N_QUBITS = 13
N_LAYERS = 8
DIM = 2 ** N_QUBITS          # 8192
N_PARAMS = 208
BLK = 512
NPAIR = 8                    # block pairs per core
STREAM_COLS = NPAIR * 4096 + 2048   # 34816
NSLOT = 4 * NPAIR + 4        # stationary slots (pairs*4k + single*4k)

_CACHE = {}


def _assignment():
    """64 same-row block pairs + 8 singles; core c gets pairs[8c:8c+8] and
    singles[c] -- every core moves exactly 8.5 MiB and runs the same
    instruction schedule."""
    pairs, singles = [], []
    for R in range(16):
        cs = list(range(R, 16))
        while len(cs) >= 2:
            pairs.append((R, cs.pop(0), cs.pop(0)))
        if cs:
            singles.append((R, cs[0]))
    assert len(pairs) == 64 and len(singles) == 8
    return pairs, singles


PAIRS, SINGLES = _assignment()


# ----------------------------------------------------------------- host consts

def _cx_chain_perm(nq, pairs):
    dim = 2 ** nq
    P = np.eye(dim)
    for (c, t) in pairs:
        M = np.zeros((dim, dim))
        for i in range(dim):
            bc = (i >> (nq - 1 - c)) & 1
            j = i ^ ((1 << (nq - 1 - t)) if bc else 0)
            M[j, i] = 1.0
        P = M @ P
    return P


def _host_consts():
    H1 = np.array([[1, 1], [1, -1]], dtype=np.float64) / np.sqrt(2.0)

    def kron_n(n):
        out = np.array([[1.0]])
        for _ in range(n):
            out = np.kron(out, H1)
        return out

    H7 = kron_n(7)
    H6 = kron_n(6)
    P7 = _cx_chain_perm(7, [(q, q + 1) for q in range(6)])
    Pf = _cx_chain_perm(6, [(q, q + 1) for q in range(5)])

    # fp16 matmul-operand constants:
    # cols 0:128 h7 | 128:256 p7t | 256:320 h6 | 320:384 h6pf | 384:448 pft
    # | 448:512 i64 | 512:640 i128 | 640:704 e0
    ch = np.zeros((128, 704), dtype=np.float64)
    ch[:, 0:128] = H7
    ch[:, 128:256] = P7.T
    ch[0:64, 256:320] = H6
    ch[0:64, 320:384] = H6 @ Pf
    ch[0:64, 384:448] = Pf.T
    ch[0:64, 448:512] = np.eye(64)
    ch[:, 512:640] = np.eye(128)
    ch[0, 640] = 1.0

    # f32 constants: sgn7 (rows 0:7, cols 0:128), sgn6 zero-padded to 128
    # output partitions (rows 0:6, cols 128:256), mo (col 256); cols
    # 257:273 thetas qubits 0-6 (rows 0:7), 273:289 qubits 7-12 (rows 0:6)
    cf = np.zeros((128, 289), dtype=np.float64)
    for q in range(7):
        for p in range(128):
            b = (p >> (6 - q)) & 1
            cf[q, p] = (1.0 if b else -1.0) * 0.5 / (2.0 * math.pi)
    for q in range(6):
        for f in range(64):
            b = (f >> (5 - q)) & 1
            cf[q, 128 + f] = (1.0 if b else -1.0) * 0.5 / (2.0 * math.pi)
    cf[:, 256] = np.arange(128) % 2
    return {"cons_h": np.ascontiguousarray(ch, dtype=np.float16),
            "cons_f": np.ascontiguousarray(cf, dtype=np.float32)}


# ------------------------------------------------------------------ the kernel

def _trig_tables(nc, tc, pool, phis_psum, npart, tag, width=32):
    """From r = phi/(2 pi) in PSUM [npart, 16] build SIN, COS, NSIN tables
    (sbuf [npart, 16]) of phi, robust to either cast-rounding semantics."""
    k_i = pool.tile([npart, width], mybir.dt.int32, tag=f"{tag}ki")
    nc.vector.tensor_copy(k_i[:, :], phis_psum)                  # cast f32->i32
    k_f = pool.tile([npart, width], F32, tag=f"{tag}kf")
    nc.vector.tensor_copy(k_f[:, :], k_i[:, :])                  # cast back
    frac = pool.tile([npart, width], F32, tag=f"{tag}fr")
    nc.vector.tensor_tensor(frac[:, :], phis_psum, k_f[:, :], AL.subtract)
    # frac in (-1, 1);  phi == 2*pi*frac (mod 2*pi)
    sh = pool.tile([npart, width], F32, tag=f"{tag}sh")             # sin(pi f)
    nc.scalar.activation(sh[:, :], frac[:, :],
                         mybir.ActivationFunctionType.Sin, scale=math.pi)
    u2 = pool.tile([npart, width], F32, tag=f"{tag}u2")             # sin(pi f/2)
    nc.scalar.activation(u2[:, :], frac[:, :],
                         mybir.ActivationFunctionType.Sin, scale=math.pi / 2)
    ch = pool.tile([npart, width], F32, tag=f"{tag}ch")             # cos(pi f)
    nc.vector.scalar_tensor_tensor(ch[:, :], u2[:, :], -2.0, u2[:, :],
                                   AL.mult, AL.mult)
    nc.vector.tensor_scalar_add(ch[:, :], ch[:, :], 1.0)
    sin = pool.tile([npart, width], F32, tag=f"{tag}sin")           # sin(2 pi f)
    nc.vector.scalar_tensor_tensor(sin[:, :], sh[:, :], 2.0, ch[:, :],
                                   AL.mult, AL.mult)
    cos = pool.tile([npart, width], F32, tag=f"{tag}cos")           # cos(2 pi f)
    nc.vector.scalar_tensor_tensor(cos[:, :], sh[:, :], -2.0, sh[:, :],
                                   AL.mult, AL.mult)
    nc.vector.tensor_scalar_add(cos[:, :], cos[:, :], 1.0)
    nsin = pool.tile([npart, width], F32, tag=f"{tag}ns")
    nc.vector.tensor_scalar_mul(nsin[:, :], sin[:, :], -1.0)
    return sin, cos, nsin


def _ap(x):
    return x if isinstance(x, bass.AP) else x[:, :]


def _build_program():
    nc = bacc.Bacc("TRN2", target_bir_lowering=False, debug=False,
                   num_devices=N_CORES)

    obs = nc.dram_tensor("obs", [128, STREAM_COLS], F16, kind="ExternalInput")
    cons_h = nc.dram_tensor("cons_h", [128, 704 + NSLOT], F16,
                            kind="ExternalInput")
    cons_f = nc.dram_tensor("cons_f", [128, 289], F32, kind="ExternalInput")

    vout = nc.dram_tensor("vout", [2, NPAIR * 1024 + BLK], F32,
                          kind="ExternalOutput")
    psi_re = nc.dram_tensor("psi_re", [DIM], F32, kind="ExternalOutput")
    psi_im = nc.dram_tensor("psi_im", [DIM], F32, kind="ExternalOutput")

    from contextlib import ExitStack
    with tile.TileContext(nc) as tc, ExitStack() as es:
        cpool = es.enter_context(tc.tile_pool(name="consts", bufs=1))
        wpool = es.enter_context(tc.tile_pool(name="wts", bufs=3))
        spool = es.enter_context(tc.tile_pool(name="state", bufs=3))
        opool = es.enter_context(tc.tile_pool(name="otiles", bufs=1))
        es_ps = ExitStack()   # circuit PSUM pools; closed before the matvec pool
        ppool_bld = es_ps.enter_context(tc.tile_pool(name="psbld", bufs=1, space="PSUM"))
        ppool_st = es_ps.enter_context(tc.tile_pool(name="psst", bufs=2, space="PSUM"))

        # ---- two merged control DMAs first (sel packed into cons_h; the
        #      per-call params packed into cons_f rows 0:7 / 32:38)
        ch = cpool.tile([128, 704 + NSLOT], F16, tag="cons_h")
        nc.sync.dma_start(ch[:, :], cons_h.ap())
        cf = cpool.tile([128, 289], F32R, tag="cons_f")
        nc.sync.dma_start(cf[:, :], cons_f.ap().bitcast(F32R))
        sel_sb = ch[0:64, 704:704 + NSLOT]
        thp = cf[0:7, 257:273]
        thf = cf[0:6, 273:289]

        obs_t = []
        for g in range(NPAIR):
            ot = opool.tile([128, 4096], F16, tag=f"obs{g}")
            nc.sync.dma_start(ot[:, :], obs.ap()[:, 4096 * g:4096 * (g + 1)])
            obs_t.append(ot)
        ot = opool.tile([128, 2048], F16, tag="obs_s")
        nc.sync.dma_start(ot[:, :], obs.ap()[:, NPAIR * 4096:STREAM_COLS])
        obs_t.append(ot)

        # ---- named views
        h7 = ch[:, 0:128]
        p7t = ch[:, 128:256]
        h6 = ch[0:64, 256:320]
        h6pf = ch[0:64, 320:384]
        pft = ch[0:64, 384:448]
        i64 = ch[0:64, 448:512]
        i128 = ch[:, 512:640]
        a0 = ch[:, 640:704]
        sgn7 = cf[0:7, 0:128]
        sgn6 = cf[0:6, 128:256]
        mo_ap = cf[:, 256:257].bitcast(F32)

        # ---- theta -> phase tables
        phi_ps = ppool_bld.tile([128, 32], F32, tag="bld")
        nc.tensor.matmul(phi_ps[:, 0:16], sgn7, thp, start=True, stop=True)
        nc.tensor.matmul(phi_ps[:, 16:32], sgn6, thf, start=True,
                         stop=True, skip_group_check=True)
        SIN_T, COS_T, NSIN_T = _trig_tables(nc, tc, cpool, phi_ps[:, :], 128, "t")
        SIN_P, COS_P, NSIN_P = SIN_T, COS_T, NSIN_T
        SIN_F = SIN_T[0:64, 16:32]
        COS_F = COS_T[0:64, 16:32]
        NSIN_F = NSIN_T[0:64, 16:32]

        # persistent per-layer weight tiles
        RAW = [cpool.tile([128, 512], F16, tag=f"RAW{k}", name=f"RAW{k}")
               for k in range(N_LAYERS)]
        RBW = [cpool.tile([64, 512], F16, tag=f"RBW{k}", name=f"RBW{k}")
               for k in range(N_LAYERS)]

        def wrap_rhs(t):
            return t[:, 256:512]

        wrap_rhs2 = wrap_rhs

        def build_layer(k):
            cxp = COS_P[:, 2 * k:2 * k + 1]
            sxp = SIN_P[:, 2 * k:2 * k + 1]
            czp = COS_P[:, 2 * k + 1:2 * k + 2]
            szp = SIN_P[:, 2 * k + 1:2 * k + 2]
            nszp = NSIN_P[:, 2 * k + 1:2 * k + 2]
            cxf = COS_F[:, 2 * k:2 * k + 1]
            sxf = SIN_F[:, 2 * k:2 * k + 1]
            czf = COS_F[:, 2 * k + 1:2 * k + 2]
            szf = SIN_F[:, 2 * k + 1:2 * k + 2]
            nszf = NSIN_F[:, 2 * k + 1:2 * k + 2]

            # ---- RAW_k = [RAre | RAim | -RAim | RAre],  RA = U_P^T
            LCS = wpool.tile([128, 256], F16, tag="LCS")
            nc.vector.tensor_scalar_mul(LCS[:, 0:128], h7, cxp)
            nc.vector.tensor_scalar_mul(LCS[:, 128:256], h7, sxp)
            # M = H D H is symmetric, so lhsT=h7 computes [M1|M2] in one
            # instruction (single PSUM writer; readers follow => race-free)
            Mps = ppool_bld.tile([128, 256], F32, tag="bld")
            nc.tensor.matmul(Mps[0:128, 0:256], h7, LCS[:, :],
                             start=True, stop=True)
            M1ps, M2ps = Mps[0:128, 0:128], Mps[0:128, 128:256]
            t1 = wpool.tile([128, 128], F16, tag="t1")
            nc.scalar.mul(t1[:, :], M1ps, czp)
            t2 = wpool.tile([128, 128], F16, tag="t2")
            nc.scalar.mul(t2[:, :], M1ps, szp)
            Y = wpool.tile([128, 256], F16, tag="Y")
            nc.vector.scalar_tensor_tensor(Y[:, 0:128], M2ps, nszp, t1[:, :],
                                           AL.mult, AL.add)
            nc.vector.scalar_tensor_tensor(Y[:, 128:256], M2ps, czp, t2[:, :],
                                           AL.mult, AL.add)
            TRP = ppool_bld.tile([128, 256], F16, tag="bldt")
            nc.tensor.transpose(TRP[0:128, 0:128], Y[:, 0:128], p7t)
            nc.tensor.transpose(TRP[0:128, 128:256], Y[:, 128:256], p7t)
            raw = RAW[k]
            # single wide copy depends on BOTH transposes (bank-race-free)
            nc.scalar.copy(raw[:, 0:256], TRP[0:128, 0:256])
            nc.vector.tensor_scalar_mul(raw[:, 256:384], raw[:, 128:256], -1.0)
            nc.vector.tensor_copy(raw[:, 384:512], raw[:, 0:128])

            # ---- RBW_k = [RBre | RBim | RBdre | RBdim | -RBim | RBre |
            #               -RBdim | RBdre]
            rhsF = h6 if k == 0 else h6pf
            idF = pft if k == N_LAYERS - 1 else i64
            LCSf = wpool.tile([64, 128], F16, tag="LCSf")
            nc.vector.tensor_scalar_mul(LCSf[:, 0:64], h6, cxf)
            nc.vector.tensor_scalar_mul(LCSf[:, 64:128], h6, sxf)
            Mfps = ppool_bld.tile([64, 128], F32, tag="bldf")
            nc.tensor.matmul(Mfps[0:64, 0:64], LCSf[:, 0:64], rhsF,
                             start=True, stop=True)
            nc.tensor.matmul(Mfps[0:64, 64:128], LCSf[:, 64:128], rhsF,
                             start=True, stop=True, skip_group_check=True)
            Mf = wpool.tile([64, 128], F16, tag="Mf")
            nc.scalar.copy(Mf[:, :], Mfps[0:64, 0:128])   # waits on both mms
            t1f = wpool.tile([64, 64], F16, tag="t1f")
            nc.scalar.mul(t1f[:, :], Mf[:, 0:64], czf)
            t2f = wpool.tile([64, 64], F16, tag="t2f")
            nc.scalar.mul(t2f[:, :], Mf[:, 0:64], szf)
            YF = wpool.tile([64, 128], F16, tag="YF")
            nc.vector.scalar_tensor_tensor(YF[:, 0:64], Mf[:, 64:128], nszf,
                                           t1f[:, :], AL.mult, AL.add)
            nc.vector.scalar_tensor_tensor(YF[:, 64:128], Mf[:, 64:128], czf,
                                           t2f[:, :], AL.mult, AL.add)
            TRF = ppool_bld.tile([64, 128], F16, tag="bldtf")
            nc.tensor.transpose(TRF[0:64, 0:64], YF[:, 0:64], idF)
            nc.tensor.transpose(TRF[0:64, 64:128], YF[:, 64:128], idF)
            rbw = RBW[k]
            # CX67 flip on the free axis: half-swap (f ^ 32) for k<7, the
            # folded layer-8 variant is a full reversal.
            if k < N_LAYERS - 1:
                def fv(t):   # [64, 2, 32] view with the 32-col halves swapped
                    v = t.rearrange("p (b x) -> p b x", b=2)
                    return v[:, ::-1, :]

                def dv(t):
                    return t.rearrange("p (b x) -> p b x", b=2)
            else:
                def fv(t):
                    return t[:, ::-1]

                def dv(t):
                    return t
            # single wide copy depends on BOTH transposes (bank-race-free)
            nc.scalar.copy(rbw[:, 0:128], TRF[0:64, 0:128])
            nc.vector.tensor_tensor(dv(rbw[:, 128:192]), fv(rbw[:, 0:64]),
                                    dv(rbw[:, 0:64]), AL.subtract)
            nc.vector.tensor_tensor(dv(rbw[:, 192:256]), fv(rbw[:, 64:128]),
                                    dv(rbw[:, 64:128]), AL.subtract)
            nc.vector.tensor_scalar_mul(rbw[:, 256:320], rbw[:, 64:128], -1.0)
            nc.vector.tensor_copy(rbw[:, 320:384], rbw[:, 0:64])
            nc.vector.tensor_scalar_mul(rbw[:, 384:448], rbw[:, 192:256], -1.0)
            nc.vector.tensor_copy(rbw[:, 448:512], rbw[:, 128:192])

        def chain_layer(k, a_cur, b_cur):
            raw, rbw = RAW[k], RBW[k]
            psA = ppool_st.tile([64, 256], F32, tag="stA")
            nc.tensor.matmul(psA[:, :], _ap(a_cur), raw[:, 0:256],
                             start=True, stop=True)
            if k > 0:
                nc.tensor.matmul(psA[:, :], _ap(b_cur), raw[:, 256:512],
                                 start=False, stop=False, skip_group_check=True)
            T1 = spool.tile([64, 256], F16, tag="T1")
            nc.scalar.copy(T1[:, 0:128], psA[:, 0:128])
            nc.vector.tensor_copy(T1[:, 128:256], psA[:, 128:256])

            # psB = [re | im | re_d | im_d]
            psB = ppool_st.tile([128, 256], F32, tag="stB")
            nc.tensor.matmul(psB[:, :], T1[:, 0:128], rbw[:, 0:256],
                             start=True, stop=True)
            nc.tensor.matmul(psB[:, :], T1[:, 128:256], rbw[:, 256:512],
                             start=False, stop=False, skip_group_check=True)
            keepT = spool.tile([128, 128], F16, tag="keepT")
            nc.scalar.copy(keepT[:, :], psB[:, 0:128])
            a3 = spool.tile([128, 64], F16, tag="a3")
            nc.vector.scalar_tensor_tensor(a3[:, :], psB[:, 128:192], mo_ap,
                                           keepT[:, 0:64], AL.mult, AL.add)
            b3 = spool.tile([128, 64], F16, tag="b3")
            nc.vector.scalar_tensor_tensor(b3[:, :], psB[:, 192:256], mo_ap,
                                           keepT[:, 64:128], AL.mult, AL.add)
            return a3, b3

        # interleave: builds run ahead so PE stays busy during chain stalls
        a_cur, b_cur = a0, None
        build_layer(0)
        build_layer(1)
        for k in range(N_LAYERS):
            if k + 2 < N_LAYERS:
                build_layer(k + 2)
            a_cur, b_cur = chain_layer(k, a_cur, b_cur)

        # ---- psi -> DRAM in f32 (host uses it for the final dot)
        af = cpool.tile([128, 64], F32, tag="af")
        nc.vector.tensor_copy(af[:, :], _ap(a_cur))
        bf = cpool.tile([128, 64], F32, tag="bf")
        nc.scalar.copy(bf[:, :], _ap(b_cur))
        nc.scalar.dma_start(psi_re.ap().rearrange("(p f) -> p f", p=128),
                            af[:, :])
        nc.scalar.dma_start(psi_im.ap().rearrange("(p f) -> p f", p=128),
                            bf[:, :])

        # ---- A64[r, 64*chi + clo] = psi[128 r + 64 chi + clo]
        A64a = cpool.tile([64, 128], F16, tag="A64a")
        A64b = cpool.tile([64, 128], F16, tag="A64b")
        for comp, (st, A64) in enumerate(((a_cur, A64a), (b_cur, A64b))):
            tpsT = ppool_bld.tile([128, 256], F16, tag="bldt", name=f"tps{comp}")
            tps = tpsT[0:64, 0:128]
            nc.tensor.transpose(tps, _ap(st), i128)
            aTs = cpool.tile([64, 128], F16, tag=f"aTs{comp}")
            if comp == 0:
                nc.scalar.copy(aTs[:, :], tps)
            else:
                nc.vector.tensor_copy(aTs[:, :], tps)
            for chi in range(2):
                tr2T = ppool_bld.tile([64, 128], F16, tag="bldtf",
                                      name=f"tr2{comp}{chi}")
                tr2 = tr2T[0:64, 0:64]
                src = aTs[:, :].rearrange("p (c two) -> two p c", two=2)[chi]
                nc.tensor.transpose(tr2, src, i64)
                if chi == 0:
                    nc.scalar.copy(A64[:, 0:64], tr2)
                else:
                    nc.vector.tensor_copy(A64[:, 64:128], tr2)

        # ---- W[q, 2s+comp] = psi_comp[128*idx_s + q]  (idx via sel input)
        psWaT = ppool_st.tile([128, 256], F32, tag="stA", name="psWaT")
        psWbT = ppool_st.tile([128, 256], F32, tag="stB", name="psWbT")
        psWa, psWb = psWaT[0:128, 0:NSLOT], psWbT[0:128, 0:NSLOT]
        nc.tensor.matmul(psWa, A64a[:, :], sel_sb, start=True, stop=True)
        nc.tensor.matmul(psWb, A64b[:, :], sel_sb, start=True, stop=True,
                         skip_group_check=True)
        W = cpool.tile([128, 2 * NSLOT], F16, tag="W")
        wv = W[:, :].rearrange("p (s two) -> two p s", two=2)
        nc.vector.tensor_copy(wv[0], psWa)
        nc.vector.tensor_copy(wv[1], psWb)

        # ---- matvec: 8 pair groups ([2,1024] PSUM) + 1 single ([2,512])
        es_ps.close()   # release circuit PSUM banks
        ppool_mv = es.enter_context(tc.tile_pool(name="psmv", bufs=3,
                                                 space="PSUM"))
        ppool_mv2 = es.enter_context(tc.tile_pool(name="psmv2", bufs=1,
                                                  space="PSUM"))
        vo = cpool.tile([2, NPAIR * 1024 + BLK], F32, tag="vo")
        for g in range(NPAIR):
            PS = ppool_mv.tile([2, 1024], F32, tag="mvp")
            for k in range(4):
                wsl = W[:, 8 * g + 2 * k:8 * g + 2 * k + 2]
                for j in range(2):
                    nc.tensor.matmul(
                        PS[:, 512 * j:512 * (j + 1)], wsl,
                        obs_t[g][:, 1024 * k + 512 * j:1024 * k + 512 * (j + 1)],
                        start=(k == 0), stop=(k == 3),
                        skip_group_check=(j == 1))
            if g % 2 == 0:
                nc.scalar.copy(vo[:, 1024 * g:1024 * (g + 1)], PS[:, :])
            else:
                nc.vector.tensor_copy(vo[:, 1024 * g:1024 * (g + 1)], PS[:, :])
        PS = ppool_mv2.tile([2, BLK], F32, tag="mvs")
        for k in range(4):
            nc.tensor.matmul(PS[:, :],
                             W[:, 8 * NPAIR + 2 * k:8 * NPAIR + 2 * k + 2],
                             obs_t[NPAIR][:, BLK * k:BLK * (k + 1)],
                             start=(k == 0), stop=(k == 3))
        nc.scalar.copy(vo[:, NPAIR * 1024:NPAIR * 1024 + BLK], PS[:, :])
        nc.sync.dma_start(vout.ap(), vo[:, :])

    nc.compile()
    return nc


def _get_program():
    if "nc" not in _CACHE:
        _CACHE["nc"] = _build_program()
        _CACHE["consts"] = _host_consts()
    return _CACHE["nc"], _CACHE["consts"]


def _make_in_maps(params, observable):
    nc, consts = _get_program()
    params = np.asarray(params, dtype=np.float32)
    O = np.asarray(observable, dtype=np.float32)
    eye64 = np.eye(64, dtype=np.float16)
    # params flat layout: k*26 + h*13 + q -> th_view[q, 2k+h]
    th = params.reshape(8, 2, 13).transpose(2, 0, 1).reshape(13, 16)
    cons_f = consts["cons_f"].copy()
    cons_f[0:7, 257:273] = th[0:7]
    cons_f[0:6, 273:289] = th[7:13]
    in_maps = []

    def sblock(R, C):
        Sb = O[BLK * R:BLK * (R + 1), BLK * C:BLK * (C + 1)]
        if R != C:
            Sb = Sb + O[BLK * C:BLK * (C + 1), BLK * R:BLK * (R + 1)].T
        return Sb.astype(np.float16)

    for c in range(N_CORES):
        stream = np.empty((128, STREAM_COLS), dtype=np.float16)
        idx = []
        for g, (R, C1, C2) in enumerate(PAIRS[NPAIR * c:NPAIR * (c + 1)]):
            S1, S2 = sblock(R, C1), sblock(R, C2)
            for k in range(4):
                base = 4096 * g + 1024 * k
                stream[:, base:base + BLK] = S1[128 * k:128 * (k + 1), :]
                stream[:, base + BLK:base + 1024] = S2[128 * k:128 * (k + 1), :]
                idx.append(4 * R + k)
        R, C = SINGLES[c]
        Ss = sblock(R, C)
        for k in range(4):
            base = NPAIR * 4096 + BLK * k
            stream[:, base:base + BLK] = Ss[128 * k:128 * (k + 1), :]
            idx.append(4 * R + k)
        ch = np.zeros((128, 704 + NSLOT), dtype=np.float16)
        ch[:, 0:704] = consts["cons_h"]
        ch[0:64, 704:704 + NSLOT] = eye64[:, idx]
        in_maps.append({"cons_h": ch, "cons_f": cons_f, "obs": stream})
    return nc, in_maps


def run(params, observable, trace=False):
    nc, in_maps = _make_in_maps(params, observable)
    res = run_bass_kernel_spmd(nc, in_maps, core_ids=list(range(N_CORES)),
                               trace=trace)
    a = np.asarray(res.results[0]["psi_re"], dtype=np.float64)
    b = np.asarray(res.results[0]["psi_im"], dtype=np.float64)
    loss = 0.0
    for c in range(N_CORES):
        v = np.asarray(res.results[c]["vout"], dtype=np.float64)
        for g, (R, C1, C2) in enumerate(PAIRS[NPAIR * c:NPAIR * (c + 1)]):
            for j, C in enumerate((C1, C2)):
                sl = slice(1024 * g + BLK * j, 1024 * g + BLK * (j + 1))
                cl = slice(BLK * C, BLK * (C + 1))
                loss += v[0, sl] @ a[cl] + v[1, sl] @ b[cl]
        R, C = SINGLES[c]
        sl = slice(NPAIR * 1024, NPAIR * 1024 + BLK)
        cl = slice(BLK * C, BLK * (C + 1))
        loss += v[0, sl] @ a[cl] + v[1, sl] @ b[cl]
    return np.float32(loss), res


def kernel(params, observable):
    loss, _ = run(params, observable, trace=False)
    return np.float32(loss)



# revision 5
# speedup vs baseline: 1.0657x; 1.0657x over previous
"""Trainium2 Bass kernel for nn_AdjointCircuitModule (13-qubit HEA circuit +
dense observable expectation), SPMD across 8 NeuronCores.

Strategy (v2)
-------------
loss = <psi|O|psi> = psi^T Osym psi, Osym = (O + O^T)/2.  Only the symmetric
part matters, so the host streams the upper triangle of S = O + O^T in
512x512 blocks: 136 blocks, 17 per core (every block identical cost =>
perfectly uniform SPMD program).  Blocks are fp16 (quantization error
~3e-4 on the scalar) => 8.9 MB/core vs 32 MB full-f32.  Same-row blocks
are paired so the matvec runs 1024-col moving operands.

* Circuit: every core simulates the full 13-qubit circuit redundantly, in
  fp16 (10-bit mantissa keeps |dpsi| ~1e-3; fp16 matmuls run at 2x the
  fp32r rate and get fast-weight-load).  State held as L0 matrix S[p, f]
  (qubits 0-6 on 128 partitions, 7-12 on 64 free cols).  Per layer:
    - weight tiles RAW = [RAre|RAim|-RAim|RAre] (128x512) and
      RBW = [RBre|RBdre|RBim|RBdim|-RBim|-RBdim|RBre|RBdre] (64x512)
      built from trig tables, pipelined 2 layers ahead of the state chain,
    - state chain: psA = 2 matmuls, PSUM->SBUF copy (2 engines), psB = 2
      matmuls (the _d columns compute the CX67 column-flip difference),
      CX67 = keep-part copy + 2 scalar_tensor_tensor.
* Matvec: per group (8 block-pairs + 1 single): PSUM [2,1024] accumulates
  4 matmuls (stationary = psi rows as fp16 pairs from W, moving = the fp16
  stream tile).  Drains go to SBUF vout; one DMA returns [2, 8704] and the
  host does the final block-dot against psi (psi_re/psi_im outputs).
"""

import math

import numpy as np

import concourse.bacc as bacc
import concourse.bass as bass
import concourse.mybir as mybir
import concourse.tile as tile
from concourse.bass_utils import run_bass_kernel_spmd

F32 = mybir.dt.float32
F32R = mybir.dt.float32r
F16 = mybir.dt.float16
AL = mybir.AluOpType

N_CORES = 8
N_QUBITS = 13
N_LAYERS = 8
DIM = 2 ** N_QUBITS          # 8192
N_PARAMS = 208
BLK = 512
NPAIR = 8                    # block pairs per core
STREAM_COLS = NPAIR * 4096 + 2048   # 34816
NSLOT = 4 * NPAIR + 4        # stationary slots (pairs*4k + single*4k)

_CACHE = {}


def _assignment():
    """64 same-row block pairs + 8 singles; core c gets pairs[8c:8c+8] and
    singles[c] -- every core moves exactly 8.5 MiB and runs the same
    instruction schedule."""
    pairs, singles = [], []
    for R in range(16):
        cs = list(range(R, 16))
        while len(cs) >= 2:
            pairs.append((R, cs.pop(0), cs.pop(0)))
        if cs:
            singles.append((R, cs[0]))
    assert len(pairs) == 64 and len(singles) == 8
    return pairs, singles


PAIRS, SINGLES = _assignment()


# ----------------------------------------------------------------- host consts

def _cx_chain_perm(nq, pairs):
    dim = 2 ** nq
    P = np.eye(dim)
    for (c, t) in pairs:
        M = np.zeros((dim, dim))
        for i in range(dim):
            bc = (i >> (nq - 1 - c)) & 1
            j = i ^ ((1 << (nq - 1 - t)) if bc else 0)
            M[j, i] = 1.0
        P = M @ P
    return P


def _host_consts():
    H1 = np.array([[1, 1], [1, -1]], dtype=np.float64) / np.sqrt(2.0)

    def kron_n(n):
        out = np.array([[1.0]])
        for _ in range(n):
            out = np.kron(out, H1)
        return out

    H7 = kron_n(7)
    H6 = kron_n(6)
    P7 = _cx_chain_perm(7, [(q, q + 1) for q in range(6)])
    Pf = _cx_chain_perm(6, [(q, q + 1) for q in range(5)])

    # fp16 matmul-operand constants:
    # cols 0:128 h7 | 128:256 p7t | 256:320 h6 | 320:384 h6pf | 384:448 pft
    # | 448:512 i64 | 512:640 i128 | 640:704 e0
    ch = np.zeros((128, 704), dtype=np.float64)
    ch[:, 0:128] = H7
    ch[:, 128:256] = P7.T
    ch[0:64, 256:320] = H6
    ch[0:64, 320:384] = H6 @ Pf
    ch[0:64, 384:448] = Pf.T
    ch[0:64, 448:512] = np.eye(64)
    ch[:, 512:640] = np.eye(128)
    ch[0, 640] = 1.0

    # f32 constants: sgn7 (rows 0:7, cols 0:128), sgn6 zero-padded to 128
    # output partitions (rows 0:6, cols 128:256), mo (col 256); cols
    # 257:273 thetas qubits 0-6 (rows 0:7), 273:289 qubits 7-12 (rows 0:6)
    cf = np.zeros((128, 289), dtype=np.float64)
    for q in range(7):
        for p in range(128):
            b = (p >> (6 - q)) & 1
            cf[q, p] = (1.0 if b else -1.0) * 0.5 / (2.0 * math.pi)
    for q in range(6):
        for f in range(64):
            b = (f >> (5 - q)) & 1
            cf[q, 128 + f] = (1.0 if b else -1.0) * 0.5 / (2.0 * math.pi)
    cf[:, 256] = np.arange(128) % 2
    return {"cons_h": np.ascontiguousarray(ch, dtype=np.float16),
            "cons_f": np.ascontiguousarray(cf, dtype=np.float32)}


# ------------------------------------------------------------------ the kernel

def _trig_tables(nc, tc, pool, phis_psum, npart, tag, width=32):
    """From r = phi/(2 pi) in PSUM [npart, 16] build SIN, COS, NSIN tables
    (sbuf [npart, 16]) of phi, robust to either cast-rounding semantics."""
    k_i = pool.tile([npart, width], mybir.dt.int32, tag=f"{tag}ki")
    nc.vector.tensor_copy(k_i[:, :], phis_psum)                  # cast f32->i32
    k_f = pool.tile([npart, width], F32, tag=f"{tag}kf")
    nc.vector.tensor_copy(k_f[:, :], k_i[:, :])                  # cast back
    frac = pool.tile([npart, width], F32, tag=f"{tag}fr")
    nc.vector.tensor_tensor(frac[:, :], phis_psum, k_f[:, :], AL.subtract)
    # frac in (-1, 1);  phi == 2*pi*frac (mod 2*pi)
    sh = pool.tile([npart, width], F32, tag=f"{tag}sh")             # sin(pi f)
    nc.scalar.activation(sh[:, :], frac[:, :],
                         mybir.ActivationFunctionType.Sin, scale=math.pi)
    u2 = pool.tile([npart, width], F32, tag=f"{tag}u2")             # sin(pi f/2)
    nc.scalar.activation(u2[:, :], frac[:, :],
                         mybir.ActivationFunctionType.Sin, scale=math.pi / 2)
    ch = pool.tile([npart, width], F32, tag=f"{tag}ch")             # cos(pi f)
    nc.vector.scalar_tensor_tensor(ch[:, :], u2[:, :], -2.0, u2[:, :],
                                   AL.mult, AL.mult)
    nc.vector.tensor_scalar_add(ch[:, :], ch[:, :], 1.0)
    sin = pool.tile([npart, width], F32, tag=f"{tag}sin")           # sin(2 pi f)
    nc.vector.scalar_tensor_tensor(sin[:, :], sh[:, :], 2.0, ch[:, :],
                                   AL.mult, AL.mult)
    cos = pool.tile([npart, width], F32, tag=f"{tag}cos")           # cos(2 pi f)
    nc.vector.scalar_tensor_tensor(cos[:, :], sh[:, :], -2.0, sh[:, :],
                                   AL.mult, AL.mult)
    nc.vector.tensor_scalar_add(cos[:, :], cos[:, :], 1.0)
    nsin = pool.tile([npart, width], F32, tag=f"{tag}ns")
    nc.vector.tensor_scalar_mul(nsin[:, :], sin[:, :], -1.0)
    return sin, cos, nsin


def _ap(x):
    return x if isinstance(x, bass.AP) else x[:, :]


def _build_program():
    nc = bacc.Bacc("TRN2", target_bir_lowering=False, debug=False,
                   num_devices=N_CORES)

    obs = nc.dram_tensor("obs", [128, STREAM_COLS], F16, kind="ExternalInput")
    cons_h = nc.dram_tensor("cons_h", [128, 704 + NSLOT], F16,
                            kind="ExternalInput")
    cons_f = nc.dram_tensor("cons_f", [128, 289], F32, kind="ExternalInput")

    vout = nc.dram_tensor("vout", [2, NPAIR * 1024 + BLK], F32,
                          kind="ExternalOutput")
    psi_re = nc.dram_tensor("psi_re", [DIM], F32, kind="ExternalOutput")
    psi_im = nc.dram_tensor("psi_im", [DIM], F32, kind="ExternalOutput")

    from contextlib import ExitStack
    with tile.TileContext(nc) as tc, ExitStack() as es:
        cpool = es.enter_context(tc.tile_pool(name="consts", bufs=1))
        wpool = es.enter_context(tc.tile_pool(name="wts", bufs=3))
        spool = es.enter_context(tc.tile_pool(name="state", bufs=3))
        opool = es.enter_context(tc.tile_pool(name="otiles", bufs=1))
        es_ps = ExitStack()   # circuit PSUM pools; closed before the matvec pool
        ppool_bld = es_ps.enter_context(tc.tile_pool(name="psbld", bufs=1, space="PSUM"))
        ppool_st = es_ps.enter_context(tc.tile_pool(name="psst", bufs=2, space="PSUM"))

        # ---- ACT (Sin) table preload: the first activation pays a ~1.5us
        #      table load; run a dummy one at t=0 so the trig chain doesn't.
        warm = cpool.tile([1, 16], F32, tag="warm")
        nc.gpsimd.memset(warm[:, :], 0.0)
        warm2 = cpool.tile([1, 16], F32, tag="warm2")
        nc.scalar.activation(warm2[:, :], warm[:, :],
                             mybir.ActivationFunctionType.Sin, scale=math.pi)

        # ---- control DMAs: cf FIRST (it gates the trig tables -> builds),
        #      then ch, then the obs stream (sel packed into cons_h; per-call
        #      params packed into cons_f rows 0:7 / 32:38)
        cf = cpool.tile([128, 289], F32R, tag="cons_f")
        nc.sync.dma_start(cf[:, :], cons_f.ap().bitcast(F32R))
        ch = cpool.tile([128, 704 + NSLOT], F16, tag="cons_h")
        nc.sync.dma_start(ch[:, :], cons_h.ap())
        sel_sb = ch[0:64, 704:704 + NSLOT]
        thp = cf[0:7, 257:273]
        thf = cf[0:6, 273:289]

        obs_t = []
        for g in range(NPAIR):
            ot = opool.tile([128, 4096], F16, tag=f"obs{g}")
            nc.sync.dma_start(ot[:, :], obs.ap()[:, 4096 * g:4096 * (g + 1)])
            obs_t.append(ot)
        ot = opool.tile([128, 2048], F16, tag="obs_s")
        nc.sync.dma_start(ot[:, :], obs.ap()[:, NPAIR * 4096:STREAM_COLS])
        obs_t.append(ot)

        # ---- named views
        h7 = ch[:, 0:128]
        p7t = ch[:, 128:256]
        h6 = ch[0:64, 256:320]
        h6pf = ch[0:64, 320:384]
        pft = ch[0:64, 384:448]
        i64 = ch[0:64, 448:512]
        i128 = ch[:, 512:640]
        a0 = ch[:, 640:704]
        sgn7 = cf[0:7, 0:128]
        sgn6 = cf[0:6, 128:256]
        mo_ap = cf[:, 256:257].bitcast(F32)

        # ---- theta -> phase tables
        phi_ps = ppool_bld.tile([128, 32], F32, tag="bld")
        nc.tensor.matmul(phi_ps[:, 0:16], sgn7, thp, start=True, stop=True)
        nc.tensor.matmul(phi_ps[:, 16:32], sgn6, thf, start=True,
                         stop=True, skip_group_check=True)
        SIN_T, COS_T, NSIN_T = _trig_tables(nc, tc, cpool, phi_ps[:, :], 128, "t")
        SIN_P, COS_P, NSIN_P = SIN_T, COS_T, NSIN_T
        SIN_F = SIN_T[0:64, 16:32]
        COS_F = COS_T[0:64, 16:32]
        NSIN_F = NSIN_T[0:64, 16:32]

        # persistent per-layer weight tiles
        RAW = [cpool.tile([128, 512], F16, tag=f"RAW{k}", name=f"RAW{k}")
               for k in range(N_LAYERS)]
        RBW = [cpool.tile([64, 512], F16, tag=f"RBW{k}", name=f"RBW{k}")
               for k in range(N_LAYERS)]

        def wrap_rhs(t):
            return t[:, 256:512]

        wrap_rhs2 = wrap_rhs

        def build_layer(k):
            cxp = COS_P[:, 2 * k:2 * k + 1]
            sxp = SIN_P[:, 2 * k:2 * k + 1]
            czp = COS_P[:, 2 * k + 1:2 * k + 2]
            szp = SIN_P[:, 2 * k + 1:2 * k + 2]
            nszp = NSIN_P[:, 2 * k + 1:2 * k + 2]
            cxf = COS_F[:, 2 * k:2 * k + 1]
            sxf = SIN_F[:, 2 * k:2 * k + 1]
            czf = COS_F[:, 2 * k + 1:2 * k + 2]
            szf = SIN_F[:, 2 * k + 1:2 * k + 2]
            nszf = NSIN_F[:, 2 * k + 1:2 * k + 2]

            # ---- RAW_k = [RAre | RAim | -RAim | RAre],  RA = U_P^T
            LCS = wpool.tile([128, 256], F16, tag="LCS")
            nc.vector.tensor_scalar_mul(LCS[:, 0:128], h7, cxp)
            nc.vector.tensor_scalar_mul(LCS[:, 128:256], h7, sxp)
            # M = H D H is symmetric, so lhsT=h7 computes [M1|M2] in one
            # instruction (single PSUM writer; readers follow => race-free)
            Mps = ppool_bld.tile([128, 256], F32, tag="bld")
            nc.tensor.matmul(Mps[0:128, 0:256], h7, LCS[:, :],
                             start=True, stop=True)
            M1ps, M2ps = Mps[0:128, 0:128], Mps[0:128, 128:256]
            t1 = wpool.tile([128, 128], F16, tag="t1")
            nc.scalar.mul(t1[:, :], M1ps, czp)
            t2 = wpool.tile([128, 128], F16, tag="t2")
            nc.scalar.mul(t2[:, :], M1ps, szp)
            Y = wpool.tile([128, 256], F16, tag="Y")
            nc.vector.scalar_tensor_tensor(Y[:, 0:128], M2ps, nszp, t1[:, :],
                                           AL.mult, AL.add)
            nc.vector.scalar_tensor_tensor(Y[:, 128:256], M2ps, czp, t2[:, :],
                                           AL.mult, AL.add)
            TRP = ppool_bld.tile([128, 256], F16, tag="bldt")
            nc.tensor.transpose(TRP[0:128, 0:128], Y[:, 0:128], p7t)
            nc.tensor.transpose(TRP[0:128, 128:256], Y[:, 128:256], p7t)
            raw = RAW[k]
            # single wide copy depends on BOTH transposes (bank-race-free)
            nc.scalar.copy(raw[:, 0:256], TRP[0:128, 0:256])
            nc.vector.tensor_scalar_mul(raw[:, 256:384], raw[:, 128:256], -1.0)
            nc.vector.tensor_copy(raw[:, 384:512], raw[:, 0:128])

            # ---- RBW_k = [RBre | RBim | RBdre | RBdim | -RBim | RBre |
            #               -RBdim | RBdre]
            rhsF = h6 if k == 0 else h6pf
            idF = pft if k == N_LAYERS - 1 else i64
            LCSf = wpool.tile([64, 128], F16, tag="LCSf")
            nc.vector.tensor_scalar_mul(LCSf[:, 0:64], h6, cxf)
            nc.vector.tensor_scalar_mul(LCSf[:, 64:128], h6, sxf)
            Mfps = ppool_bld.tile([64, 128], F32, tag="bldf")
            nc.tensor.matmul(Mfps[0:64, 0:64], LCSf[:, 0:64], rhsF,
                             start=True, stop=True)
            nc.tensor.matmul(Mfps[0:64, 64:128], LCSf[:, 64:128], rhsF,
                             start=True, stop=True, skip_group_check=True)
            Mf = wpool.tile([64, 128], F16, tag="Mf")
            nc.scalar.copy(Mf[:, :], Mfps[0:64, 0:128])   # waits on both mms
            t1f = wpool.tile([64, 64], F16, tag="t1f")
            nc.scalar.mul(t1f[:, :], Mf[:, 0:64], czf)
            t2f = wpool.tile([64, 64], F16, tag="t2f")
            nc.scalar.mul(t2f[:, :], Mf[:, 0:64], szf)
            YF = wpool.tile([64, 128], F16, tag="YF")
            nc.vector.scalar_tensor_tensor(YF[:, 0:64], Mf[:, 64:128], nszf,
                                           t1f[:, :], AL.mult, AL.add)
            nc.vector.scalar_tensor_tensor(YF[:, 64:128], Mf[:, 64:128], czf,
                                           t2f[:, :], AL.mult, AL.add)
            TRF = ppool_bld.tile([64, 128], F16, tag="bldtf")
            nc.tensor.transpose(TRF[0:64, 0:64], YF[:, 0:64], idF)
            nc.tensor.transpose(TRF[0:64, 64:128], YF[:, 64:128], idF)
            rbw = RBW[k]
            # CX67 flip on the free axis: half-swap (f ^ 32) for k<7, the
            # folded layer-8 variant is a full reversal.
            if k < N_LAYERS - 1:
                def fv(t):   # [64, 2, 32] view with the 32-col halves swapped
                    v = t.rearrange("p (b x) -> p b x", b=2)
                    return v[:, ::-1, :]

                def dv(t):
                    return t.rearrange("p (b x) -> p b x", b=2)
            else:
                def fv(t):
                    return t[:, ::-1]

                def dv(t):
                    return t
            # single wide copy depends on BOTH transposes (bank-race-free)
            nc.scalar.copy(rbw[:, 0:128], TRF[0:64, 0:128])
            nc.vector.tensor_tensor(dv(rbw[:, 128:192]), fv(rbw[:, 0:64]),
                                    dv(rbw[:, 0:64]), AL.subtract)
            nc.vector.tensor_tensor(dv(rbw[:, 192:256]), fv(rbw[:, 64:128]),
                                    dv(rbw[:, 64:128]), AL.subtract)
            nc.vector.tensor_scalar_mul(rbw[:, 256:320], rbw[:, 64:128], -1.0)
            nc.vector.tensor_copy(rbw[:, 320:384], rbw[:, 0:64])
            nc.vector.tensor_scalar_mul(rbw[:, 384:448], rbw[:, 192:256], -1.0)
            nc.vector.tensor_copy(rbw[:, 448:512], rbw[:, 128:192])

        def chain_layer(k, a_cur, b_cur):
            raw, rbw = RAW[k], RBW[k]
            psA = ppool_st.tile([64, 256], F32, tag="stA")
            nc.tensor.matmul(psA[:, :], _ap(a_cur), raw[:, 0:256],
                             start=True, stop=True)
            if k > 0:
                nc.tensor.matmul(psA[:, :], _ap(b_cur), raw[:, 256:512],
                                 start=False, stop=False, skip_group_check=True)
            T1 = spool.tile([64, 256], F16, tag="T1")
            nc.scalar.copy(T1[:, 0:128], psA[:, 0:128])
            nc.vector.tensor_copy(T1[:, 128:256], psA[:, 128:256])

            # psB = [re | im | re_d | im_d]
            psB = ppool_st.tile([128, 256], F32, tag="stB")
            nc.tensor.matmul(psB[:, :], T1[:, 0:128], rbw[:, 0:256],
                             start=True, stop=True)
            nc.tensor.matmul(psB[:, :], T1[:, 128:256], rbw[:, 256:512],
                             start=False, stop=False, skip_group_check=True)
            keepT = spool.tile([128, 128], F16, tag="keepT")
            nc.scalar.copy(keepT[:, :], psB[:, 0:128])
            a3 = spool.tile([128, 64], F16, tag="a3")
            nc.vector.scalar_tensor_tensor(a3[:, :], psB[:, 128:192], mo_ap,
                                           keepT[:, 0:64], AL.mult, AL.add)
            b3 = spool.tile([128, 64], F16, tag="b3")
            nc.vector.scalar_tensor_tensor(b3[:, :], psB[:, 192:256], mo_ap,
                                           keepT[:, 64:128], AL.mult, AL.add)
            return a3, b3

        # interleave: builds run ahead so PE stays busy during chain stalls
        a_cur, b_cur = a0, None
        build_layer(0)
        build_layer(1)
        for k in range(N_LAYERS):
            if k + 2 < N_LAYERS:
                build_layer(k + 2)
            a_cur, b_cur = chain_layer(k, a_cur, b_cur)

        # ---- psi -> DRAM in f32 (host uses it for the final dot)
        af = cpool.tile([128, 64], F32, tag="af")
        nc.vector.tensor_copy(af[:, :], _ap(a_cur))
        bf = cpool.tile([128, 64], F32, tag="bf")
        nc.scalar.copy(bf[:, :], _ap(b_cur))
        nc.sync.dma_start(psi_re.ap().rearrange("(p f) -> p f", p=128),
                          af[:, :])
        nc.sync.dma_start(psi_im.ap().rearrange("(p f) -> p f", p=128),
                          bf[:, :])

        # ---- A64[r, 64*chi + clo] = psi[128 r + 64 chi + clo] = S[2r+chi, clo]
        #      pure partition-fold reshape -> SBUF->SBUF DMA (no PE/DVE time)
        A64a = cpool.tile([64, 128], F16, tag="A64a")
        A64b = cpool.tile([64, 128], F16, tag="A64b")
        for comp, (st, A64) in enumerate(((a_cur, A64a), (b_cur, A64b))):
            eng = nc.scalar if comp == 0 else nc.sync
            eng.dma_start(A64[:, 0:64], _ap(st)[0::2, :])
            eng.dma_start(A64[:, 64:128], _ap(st)[1::2, :])

        # ---- W[q, 2s+comp] = psi_comp[128*idx_s + q]  (idx via sel input)
        psWaT = ppool_st.tile([128, 256], F32, tag="stA", name="psWaT")
        psWbT = ppool_st.tile([128, 256], F32, tag="stB", name="psWbT")
        psWa, psWb = psWaT[0:128, 0:NSLOT], psWbT[0:128, 0:NSLOT]
        nc.tensor.matmul(psWa, A64a[:, :], sel_sb, start=True, stop=True)
        nc.tensor.matmul(psWb, A64b[:, :], sel_sb, start=True, stop=True,
                         skip_group_check=True)
        W = cpool.tile([128, 2 * NSLOT], F16, tag="W")
        wv = W[:, :].rearrange("p (s two) -> two p s", two=2)
        nc.vector.tensor_copy(wv[0], psWa)
        nc.vector.tensor_copy(wv[1], psWb)

        # ---- matvec: 8 pair groups ([2,1024] PSUM) + 1 single ([2,512]).
        #      Per-group drain copy then per-group DMA to DRAM (overlapped
        #      with later groups) -- no 2-partition mega-DMA tail.
        es_ps.close()   # release circuit PSUM banks
        ppool_mv = es.enter_context(tc.tile_pool(name="psmv", bufs=3,
                                                 space="PSUM"))
        ppool_mv2 = es.enter_context(tc.tile_pool(name="psmv2", bufs=1,
                                                  space="PSUM"))
        vo = cpool.tile([2, NPAIR * 1024 + BLK], F32, tag="vo")
        for g in range(NPAIR):
            PS = ppool_mv.tile([2, 1024], F32, tag="mvp")
            for k in range(4):
                wsl = W[:, 8 * g + 2 * k:8 * g + 2 * k + 2]
                for j in range(2):
                    nc.tensor.matmul(
                        PS[:, 512 * j:512 * (j + 1)], wsl,
                        obs_t[g][:, 1024 * k + 512 * j:1024 * k + 512 * (j + 1)],
                        start=(k == 0), stop=(k == 3),
                        skip_group_check=(j == 1))
            vsl = vo[:, 1024 * g:1024 * (g + 1)]
            if g % 2 == 0:
                nc.scalar.copy(vsl, PS[:, :])
            else:
                nc.vector.tensor_copy(vsl, PS[:, :])
            nc.sync.dma_start(vout.ap()[:, 1024 * g:1024 * (g + 1)], vsl)
        PS = ppool_mv2.tile([2, BLK], F32, tag="mvs")
        for k in range(4):
            nc.tensor.matmul(PS[:, :],
                             W[:, 8 * NPAIR + 2 * k:8 * NPAIR + 2 * k + 2],
                             obs_t[NPAIR][:, BLK * k:BLK * (k + 1)],
                             start=(k == 0), stop=(k == 3))
        vsl = vo[:, NPAIR * 1024:NPAIR * 1024 + BLK]
        nc.scalar.copy(vsl, PS[:, :])
        nc.sync.dma_start(vout.ap()[:, NPAIR * 1024:NPAIR * 1024 + BLK], vsl)

    nc.compile()
    return nc


def _get_program():
    if "nc" not in _CACHE:
        _CACHE["nc"] = _build_program()
        _CACHE["consts"] = _host_consts()
    return _CACHE["nc"], _CACHE["consts"]


def _make_in_maps(params, observable):
    nc, consts = _get_program()
    params = np.asarray(params, dtype=np.float32)
    O = np.asarray(observable, dtype=np.float32)
    eye64 = np.eye(64, dtype=np.float16)
    # params flat layout: k*26 + h*13 + q -> th_view[q, 2k+h]
    th = params.reshape(8, 2, 13).transpose(2, 0, 1).reshape(13, 16)
    cons_f = consts["cons_f"].copy()
    cons_f[0:7, 257:273] = th[0:7]
    cons_f[0:6, 273:289] = th[7:13]
    in_maps = []

    def sblock(R, C):
        Sb = O[BLK * R:BLK * (R + 1), BLK * C:BLK * (C + 1)]
        if R != C:
            Sb = Sb + O[BLK * C:BLK * (C + 1), BLK * R:BLK * (R + 1)].T
        return Sb.astype(np.float16)

    for c in range(N_CORES):
        stream = np.empty((128, STREAM_COLS), dtype=np.float16)
        idx = []
        for g, (R, C1, C2) in enumerate(PAIRS[NPAIR * c:NPAIR * (c + 1)]):
            S1, S2 = sblock(R, C1), sblock(R, C2)
            for k in range(4):
                base = 4096 * g + 1024 * k
                stream[:, base:base + BLK] = S1[128 * k:128 * (k + 1), :]
                stream[:, base + BLK:base + 1024] = S2[128 * k:128 * (k + 1), :]
                idx.append(4 * R + k)
        R, C = SINGLES[c]
        Ss = sblock(R, C)
        for k in range(4):
            base = NPAIR * 4096 + BLK * k
            stream[:, base:base + BLK] = Ss[128 * k:128 * (k + 1), :]
            idx.append(4 * R + k)
        ch = np.zeros((128, 704 + NSLOT), dtype=np.float16)
        ch[:, 0:704] = consts["cons_h"]
        ch[0:64, 704:704 + NSLOT] = eye64[:, idx]
        in_maps.append({"cons_h": ch, "cons_f": cons_f, "obs": stream})
    return nc, in_maps


def run(params, observable, trace=False):
    nc, in_maps = _make_in_maps(params, observable)
    res = run_bass_kernel_spmd(nc, in_maps, core_ids=list(range(N_CORES)),
                               trace=trace)
    a = np.asarray(res.results[0]["psi_re"], dtype=np.float64)
    b = np.asarray(res.results[0]["psi_im"], dtype=np.float64)
    loss = 0.0
    for c in range(N_CORES):
        v = np.asarray(res.results[c]["vout"], dtype=np.float64)
        for g, (R, C1, C2) in enumerate(PAIRS[NPAIR * c:NPAIR * (c + 1)]):
            for j, C in enumerate((C1, C2)):
                sl = slice(1024 * g + BLK * j, 1024 * g + BLK * (j + 1))
                cl = slice(BLK * C, BLK * (C + 1))
                loss += v[0, sl] @ a[cl] + v[1, sl] @ b[cl]
        R, C = SINGLES[c]
        sl = slice(NPAIR * 1024, NPAIR * 1024 + BLK)
        cl = slice(BLK * C, BLK * (C + 1))
        loss += v[0, sl] @ a[cl] + v[1, sl] @ b[cl]
    return np.float32(loss), res


def kernel(params, observable):
    loss, _ = run(params, observable, trace=False)
    return np.float32(loss)

